# revision 35
# baseline (speedup 1.0000x reference)
"""AnomalyTransformer Trainium2 kernel.

3-layer transformer encoder (d=64 -> d_model=512, N=1024 tokens, B=16),
data-parallel over batch: 8 NeuronCores x 2 batches each, weights
replicated, no collectives.  The Gaussian-prior branch of the reference
is a dead computation (never touches the output) and is skipped.

Key optimizations over the straightforward mapping:
  - QK fusion: A = (hWq)(hWk)^T = h (Wq Wk^T) h^T.  M = Wq@Wk^T is
    precomputed on host (free), eliminating the K projection entirely
    and turning the layer-0 score matmul into a K=64 contraction
    (M0 = Wq0@Wk0^T is only [64,64]).
  - Split-Z softmax-denominator fold: V tiles carry a ones column
    (layout [V[:,0:512] | 1]); the Z matmul is split into two PSUM
    tiles [Z[0:256]] and [Z[256:512] | s] so the exp row-sum s comes
    out of the same matmul stream for ~1 extra streamed column instead
    of 64 tiny [128,128,1] matmuls per batch-layer.
  - Softmax uses no max-subtraction (logits empirically bounded ~15)
    and no explicit normalization: LN(Z/s + h) == LN(Z + s*h) by
    layernorm scale invariance.
  - LayerNorm chains are fused: scalar_tensor_tensor with accum_out
    computes the residual combine AND the mean-sum in one pass; a
    second pass squares with accum (E[x^2]); relu folds into the LN2
    residual op (op0=max) reading the FFN PSUM directly.  Var via
    sum1*mu - sum2 = -N*var, then ACT Sqrt(-x/N + eps) + DVE recip.
  - All matmuls bf16 (fp32 PSUM accumulation); scores computed
    directly transposed A^T so exp(A^T) tiles feed Z = S@V as lhsT.
  - z^T / g^T for the next matmul stage via XBAR DMA transposes split
    across the sync and scalar HWDGE queues.
  - Two batches' layers emitted alternately so one batch's matmuls
    overlap the other batch's DVE/ACT chains.
  - PSUM->SBUF casts split across DVE (G^T) and ACT (V) to balance.
  - When the affine params are identity (true for this problem's
    setup_inputs) the affine/bias ops are skipped; the general path is
    kept for arbitrary inputs.
"""

import numpy as np

import concourse.bass as bass
import concourse.mybir as mybir
import concourse.tile as tile
from concourse import bacc
from concourse.bass_utils import run_bass_kernel_spmd

F32 = mybir.dt.float32
BF16 = mybir.dt.bfloat16
TRACE = False

import os
BISECT = set(os.environ.get("KBISECT", "").split(","))

D0 = 64      # input feature dim
DM = 512     # d_model
NT = 1024    # tokens per batch
NB = 2       # batches per core
NCORES = 8
DC = DM // 128   # 4 dm chunks
RT = NT // 128   # 8 token tiles per batch
HF = NT // 512   # 2 moving-operand halves
HDM = DM // 2    # 256
ISQ = 1.0 / float(np.sqrt(DM))
EPS = 1e-5
AF = mybir.ActivationFunctionType
ALU = mybir.AluOpType


def build_graph(nc, affine_identity=False, bf_zero=False):
    T = NB * NT

    d = {}
    d["xt"] = nc.declare_dram_parameter("xt", [D0, T], BF16, isOutput=False)
    d["wm0"] = nc.declare_dram_parameter("wm0", [D0, D0], BF16, isOutput=False)
    d["wv0"] = nc.declare_dram_parameter("wv0", [D0, DM], BF16, isOutput=False)
    # nws[l] = -colsum(Wf[l])/DM, for the folded-LN1 FFN mean correction
    d["nws"] = nc.declare_dram_parameter("nws", [3, DM], BF16, isOutput=False)
    for nm, L in (("wms", 2), ("wvs", 2), ("wf", 3)):
        d[nm] = nc.declare_dram_parameter(nm, [L, DM, DM], BF16, isOutput=False)
    for nm in ("g1", "b1", "g2", "b2", "bf"):
        d[nm] = nc.declare_dram_parameter(nm, [3, DM], F32, isOutput=False)
    d["out"] = nc.declare_dram_parameter("out", [T, DM], F32, isOutput=True)

    with tile.TileContext(nc) as tc:
        _build_tc(tc, nc, d, affine_identity, bf_zero)
    nc.compile()
    return nc


def _build_tc(tc, nc, d, affine_identity=False, bf_zero=False):
    from contextlib import ExitStack
    ctx = ExitStack()
    with ctx:
        const = ctx.enter_context(tc.tile_pool(name="const", bufs=1))
        wpool = ctx.enter_context(tc.tile_pool(name="wpool", bufs=4))
        lnpool = ctx.enter_context(tc.tile_pool(name="lnpool", bufs=8))
        rows = ctx.enter_context(tc.tile_pool(name="rows", bufs=48))
        tchunk = ctx.enter_context(tc.tile_pool(name="tchunk", bufs=4))
        # epool/vpool/gtpool sized so BOTH batches' attention tiles can be
        # live at once -- lets the PE run batch b1's projection/attention
        # matmuls while batch b0's LN1 chain drains on DVE
        gtpool = ctx.enter_context(tc.tile_pool(name="gtpool", bufs=9))
        vpool = ctx.enter_context(tc.tile_pool(name="vpool", bufs=16))
        epool = ctx.enter_context(tc.tile_pool(name="epool", bufs=15))
        xpool = ctx.enter_context(tc.tile_pool(name="xpool", bufs=2))
        small = ctx.enter_context(tc.tile_pool(name="small", bufs=6))
        zppool = ctx.enter_context(tc.tile_pool(name="zppool", bufs=4))
        gout = ctx.enter_context(tc.tile_pool(name="gout", bufs=2))
        ps_at = ctx.enter_context(tc.tile_pool(name="ps_at", bufs=2, space="PSUM"))
        ps_mm = ctx.enter_context(tc.tile_pool(name="ps_mm", bufs=2, space="PSUM"))
        ps_z = ctx.enter_context(tc.tile_pool(name="ps_z", bufs=4, space="PSUM"))

        eps_t = const.tile([128, 1], F32)
        nc.vector.memset(eps_t, EPS)
        ones = const.tile([128, 1], BF16)
        nc.vector.memset(ones, 1.0)

        w0 = {}
        for name, shp in (("wm0", [D0, D0]), ("wv0", [D0, DM])):
            t = const.tile(shp, BF16, tag=name, name=name)
            nc.sync.dma_start(out=t, in_=d[name][:])
            w0[name] = t

        def load_w(key, idx):
            # sync-engine HWDGE queue: keeps weight prefetch off the
            # scalar queue so it never blocks ACT compute
            t = wpool.tile([128, DC, DM], BF16, tag="W", name="W")
            nc.sync.dma_start(
                out=t, in_=d[key][idx].rearrange("(c p) o -> p c o", p=128))
            return t

        def load_ln(name, l):
            t = lnpool.tile([128, DM], F32, tag="ln", name="lnw")
            nc.sync.dma_start(
                out=t, in_=d[name][l].unsqueeze(0).to_broadcast((128, DM)))
            return t

        def ln_tail(zpre, sum1ap, out_ap, gb, bb):
            """per-row LN: zpre bf16 SBUF, sum1ap [128,1] = rowsum(zpre)."""
            scr = zppool.tile([128, DM], BF16, tag="scr", name="scr")
            sum2 = small.tile([128, 1], F32, tag="sum2", name="sum2")
            nc.vector.scalar_tensor_tensor(
                out=scr, in0=zpre, scalar=0.0, in1=zpre,
                op0=ALU.bypass, op1=ALU.mult, accum_out=sum2)
            mu = small.tile([128, 1], F32, tag="mu", name="mu")
            nc.vector.tensor_scalar_mul(out=mu, in0=sum1ap, scalar1=1.0 / DM)
            nv = small.tile([128, 1], F32, tag="nv", name="nv")
            nc.vector.scalar_tensor_tensor(
                out=nv, in0=sum1ap, scalar=mu, in1=sum2,
                op0=ALU.mult, op1=ALU.subtract)
            stdv = small.tile([128, 1], F32, tag="stdv", name="stdv")
            nc.scalar.activation(out=stdv, in_=nv, func=AF.Sqrt,
                                 bias=eps_t, scale=-1.0 / DM)
            rstd = small.tile([128, 1], F32, tag="rstd", name="rstd")
            nc.vector.reciprocal(out=rstd, in_=stdv)
            nc.vector.tensor_scalar(
                out=out_ap, in0=zpre, scalar1=mu, scalar2=rstd,
                op0=ALU.subtract, op1=ALU.mult)
            if gb is not None:
                nc.vector.tensor_mul(out=out_ap, in0=out_ap, in1=gb)
                nc.vector.tensor_add(out=out_ap, in0=out_ap, in1=bb)

        xts = []
        for b in range(NB):
            xt = xpool.tile([D0, NT], BF16, tag="xt", name="xt")
            nc.sync.dma_start(out=xt, in_=d["xt"][:, b * NT:(b + 1) * NT])
            xts.append(xt)
        hT = [None] * NB    # [128, DC, NT] bf16 per batch
        h = [None] * NB     # RT row tiles [128, DM] bf16 per batch
        zs = [None] * NB
        lnp = [None] * 3

        # fold-LN1: defer LN1's normalization into the FFN matmul via LN's
        # per-row shift+scale invariance -- only valid with identity affine
        # and zero FFN bias
        fold = affine_identity and bf_zero

        for l in range(3):
            lw = {}
            if l > 0:
                lw["wm"] = load_w("wms", l - 1)
                lw["wv"] = load_w("wvs", l - 1)
            lw["wf"] = load_w("wf", l)
            if fold:
                nws = wpool.tile([1, DM], BF16, tag="nws", name="nws", bufs=2)
                nc.sync.dma_start(out=nws, in_=d["nws"][l].unsqueeze(0))
            if affine_identity:
                g1b = b1b = g2b = b2b = None
            else:
                g1b = load_ln("g1", l)
                b1b = load_ln("b1", l)
                g2b = load_ln("g2", l)
                b2b = load_ln("b2", l)
            bfb = None if bf_zero else load_ln("bf", l)

            # ---------- phase 1 per batch: G^T, V, A^T/exp, Z + LN1 ----
            for b in range(NB):
                if l == 0:
                    # G0^T = M0^T @ x^T : [64, NT]
                    gt0 = gtpool.tile([D0, NT], BF16, tag="gt0", name="gt0",
                                      bufs=3)
                    for hf in range(HF):
                        ps = ps_mm.tile([D0, 512], F32, tag="mm", name="ps")
                        nc.tensor.matmul(
                            ps, w0["wm0"],
                            xts[b][:, hf * 512:(hf + 1) * 512],
                            start=True, stop=True)
                        nc.vector.tensor_copy(
                            out=gt0[:, hf * 512:(hf + 1) * 512], in_=ps)
                    gts = None
                    v = [vpool.tile([128, DM], BF16, tag="vr", name=f"v{r}")
                         for r in range(RT)]
                    for r in range(RT):
                        ps = ps_mm.tile([128, 512], F32, tag="mm", name="ps")
                        nc.tensor.matmul(
                            ps, xts[b][:, r * 128:(r + 1) * 128], w0["wv0"],
                            start=True, stop=True)
                        nc.scalar.activation(out=v[r], in_=ps, func=AF.Copy)
                else:
                    # G^T chunks: [128, NT] x4, via lhsT = M chunks.
                    # hf pairs share the stationary operand (one LDWEIGHTS
                    # per (o,i) instead of two).
                    gts = [gtpool.tile([128, NT], BF16, tag="gt", name=f"gt{o}")
                           for o in range(DC)]
                    for o in range(DC):
                        pss = [ps_mm.tile([128, 512], F32, tag="mm",
                                          name=f"ps{hf}") for hf in range(HF)]
                        for i in range(DC):
                            for hf in range(HF):
                                nc.tensor.matmul(
                                    pss[hf],
                                    lw["wm"][:, i, o * 128:(o + 1) * 128],
                                    hT[b][:, i, hf * 512:(hf + 1) * 512],
                                    start=(i == 0), stop=(i == DC - 1))
                        for hf in range(HF):
                            nc.vector.tensor_copy(
                                out=gts[o][:, hf * 512:(hf + 1) * 512],
                                in_=pss[hf])
                    # V rows with a trailing ones column for the softmax
                    # denominator fold
                    v = [vpool.tile([128, DM + 1], BF16, tag="vr1", name=f"v{r}")
                         for r in range(RT)]
                    for r in range(RT):
                        ps = ps_mm.tile([128, 512], F32, tag="mm", name="ps")
                        for i in range(DC):
                            nc.tensor.matmul(
                                ps,
                                hT[b][:, i, r * 128:(r + 1) * 128],
                                lw["wv"][:, i, :],
                                start=(i == 0), stop=(i == DC - 1))
                        nc.scalar.activation(out=v[r][:, 0:DM], in_=ps,
                                             func=AF.Copy)
                        nc.gpsimd.memset(v[r][:, DM:DM + 1], 1.0)

                eT = []
                for c in range(RT):
                    et = epool.tile([128, NT], BF16, tag="et", name="et")
                    ats = [ps_at.tile([128, 512], F32, tag="at", name=f"at{hf}")
                           for hf in range(HF)]
                    if l == 0:
                        for hf in range(HF):
                            nc.tensor.matmul(
                                ats[hf],
                                xts[b][:, c * 128:(c + 1) * 128],
                                gt0[:, hf * 512:(hf + 1) * 512],
                                start=True, stop=True)
                    else:
                        # hf pairs share lhsT (hT chunk) per i step
                        for i in range(DC):
                            for hf in range(HF):
                                nc.tensor.matmul(
                                    ats[hf],
                                    hT[b][:, i, c * 128:(c + 1) * 128],
                                    gts[i][:, hf * 512:(hf + 1) * 512],
                                    start=(i == 0), stop=(i == DC - 1))
                    for hf in range(HF):
                        nc.scalar.activation(
                            out=et[:, hf * 512:(hf + 1) * 512], in_=ats[hf],
                            func=AF.Exp, scale=ISQ)
                    eT.append(et)

                z = [rows.tile([128, DM], BF16, tag="row", name=f"z{r}")
                     for r in range(RT)]
                for r in range(RT):
                    if fold:
                        # z[r] holds UNNORMALIZED zpre; LN1 is folded into
                        # the FFN matmul (mean via mu^T aug step, scale
                        # cancels in LN2 by scale invariance)
                        zpre = z[r]
                        acc1 = None
                    else:
                        zpre = zppool.tile([128, DM], BF16, tag="zpre",
                                           name="zpre", bufs=6)
                        acc1 = small.tile([128, 1], F32, tag="acc1",
                                          name="acc1")
                    if l == 0:
                        zp = ps_z.tile([128, DM], F32, tag="z", name="zp")
                        for c in range(RT):
                            nc.tensor.matmul(
                                zp, eT[c][:, r * 128:(r + 1) * 128], v[c],
                                start=(c == 0), stop=(c == RT - 1))
                        # PSUM->SBUF cast (+ mean-sum) on ACT
                        nc.scalar.activation(out=zpre, in_=zp, func=AF.Copy,
                                             accum_out=acc1)
                    else:
                        # zA = [Z[:,256:512] | s], zB = Z[:,0:256]
                        zA = ps_z.tile([128, HDM + 1], F32, tag="z",
                                       name="zA")
                        zB = ps_z.tile([128, HDM], F32, tag="z", name="zB")
                        for c in range(RT):
                            lt = eT[c][:, r * 128:(r + 1) * 128]
                            nc.tensor.matmul(
                                zA, lt, v[c][:, HDM:DM + 1],
                                start=(c == 0), stop=(c == RT - 1))
                            nc.tensor.matmul(
                                zB, lt, v[c][:, 0:HDM],
                                start=(c == 0), stop=(c == RT - 1))
                        s_sb = small.tile([128, 1], F32, tag="ssb",
                                          name="s_sb")
                        nc.scalar.activation(out=s_sb,
                                             in_=zA[:, HDM:HDM + 1],
                                             func=AF.Copy)
                        if acc1 is not None:
                            acc1b = small.tile([128, 1], F32, tag="acc1b",
                                               name="acc1b")
                        # zpre = s*h + Z  (LN-equivalent to Z/s + h)
                        nc.vector.scalar_tensor_tensor(
                            out=zpre[:, 0:HDM], in0=h[b][r][:, 0:HDM],
                            scalar=s_sb, in1=zB,
                            op0=ALU.mult, op1=ALU.add, accum_out=acc1)
                        nc.vector.scalar_tensor_tensor(
                            out=zpre[:, HDM:DM], in0=h[b][r][:, HDM:DM],
                            scalar=s_sb, in1=zA[:, 0:HDM],
                            op0=ALU.mult, op1=ALU.add,
                            accum_out=(acc1b if acc1 is not None else None))
                        if acc1 is not None:
                            nc.vector.tensor_add(out=acc1, in0=acc1,
                                                 in1=acc1b)
                    if not fold:
                        ln_tail(zpre, acc1, z[r], g1b, b1b)
                zs[b] = z
                lnp[l] = (g1b, b1b, g2b, b2b, bfb)

            # ---------- z^T DMA transposes, both batches ----------
            # even rows -> sync queue, odd rows -> scalar queue: halves the
            # per-queue serial time so the FFN is fed sooner
            zTs = [None] * NB
            muTs = [None] * NB
            for b in range(NB):
                zT = tchunk.tile([128, DC, NT], BF16, tag="tchunk", name="zT")
                for r in range(RT):
                    eng = nc.sync if (r % 2 == b % 2) else nc.scalar
                    eng.dma_start_transpose(
                        out=zT[:, :, r * 128:(r + 1) * 128], in_=zs[b][r])
                zTs[b] = zT
                if fold:
                    # mu^T[t] = N*mean(zpre[t,:]) via ones-matmul on zpre^T
                    muT = small.tile([1, NT], BF16, tag="muT", name="muT",
                                     bufs=2)
                    for hf in range(HF):
                        mt = ps_z.tile([1, 512], F32, tag="z", name="mt")
                        for i in range(DC):
                            nc.tensor.matmul(
                                mt, ones,
                                zT[:, i, hf * 512:(hf + 1) * 512],
                                start=(i == 0), stop=(i == DC - 1))
                        nc.scalar.activation(
                            out=muT[:, hf * 512:(hf + 1) * 512], in_=mt,
                            func=AF.Copy)
                    muTs[b] = muT

            # ---------- phase 2 per batch: FFN + LN2 (+ g^T / out) ----
            for b in range(NB):
                zT = zTs[b]
                z = zs[b]
                g1b, b1b, g2b, b2b, bfb = lnp[l]
                if l < 2:
                    g = [rows.tile([128, DM], BF16, tag="row", name=f"g{r}")
                         for r in range(RT)]
                else:
                    g = [gout.tile([128, DM], F32, tag="gout", name=f"g{r}")
                         for r in range(RT)]
                for r in range(RT):
                    fp = ps_mm.tile([128, DM], F32, tag="mm", name="fp")
                    for i in range(DC):
                        nc.tensor.matmul(
                            fp,
                            zT[:, i, r * 128:(r + 1) * 128],
                            lw["wf"][:, i, :],
                            start=(i == 0),
                            stop=(i == DC - 1 and not fold))
                    if fold:
                        # fp += mu^T x (-colsum(Wf)/N): completes
                        # fp = (zpre - mu) @ Wf
                        nc.tensor.matmul(
                            fp, muTs[b][:, r * 128:(r + 1) * 128], nws,
                            start=False, stop=True)
                    if bfb is not None:
                        nc.vector.tensor_add(out=fp, in0=fp, in1=bfb)
                    gpre = zppool.tile([128, DM], BF16, tag="zpre",
                                       name="gpre", bufs=4)
                    acc1 = small.tile([128, 1], F32, tag="acc1", name="acc1")
                    # gpre = relu(fp) + z ; acc1 = rowsum(gpre)
                    nc.vector.scalar_tensor_tensor(
                        out=gpre, in0=fp, scalar=0.0, in1=z[r],
                        op0=ALU.max, op1=ALU.add, accum_out=acc1)
                    ln_tail(gpre, acc1, g[r], g2b, b2b)
                    if l == 2:
                        nc.sync.dma_start(
                            out=d["out"][b * NT + r * 128:
                                         b * NT + (r + 1) * 128, :],
                            in_=g[r])

                if l < 2:
                    nhT = tchunk.tile([128, DC, NT], BF16, tag="tchunk",
                                      name="hT")
                    for r in range(RT):
                        eng = nc.scalar if (r % 2 == b % 2) else nc.sync
                        eng.dma_start_transpose(
                            out=nhT[:, :, r * 128:(r + 1) * 128], in_=g[r])
                    hT[b] = nhT
                    h[b] = g


def kernel(**inputs):
    x = np.asarray(inputs["x"], np.float32)          # [16, 1024, 64]
    bfdt = np.dtype(mybir.dt.np(BF16))

    def to_bf16(a):
        return np.ascontiguousarray(np.asarray(a, np.float32).astype(bfdt))

    wq0 = np.asarray(inputs["Wq0"], np.float32)
    wk0 = np.asarray(inputs["Wk0"], np.float32)
    wqs = np.asarray(inputs["Wqs"], np.float32)
    wks = np.asarray(inputs["Wks"], np.float32)
    # QK fusion: M = Wq @ Wk^T, contracted on host (fp32) then cast
    wm0 = wq0 @ wk0.T                                # [64, 64]
    wms = np.einsum("lde,lfe->ldf", wqs, wks)        # [2, 512, 512]

    wf = np.asarray(inputs["Wf"], np.float32)
    shared = {
        "wm0": to_bf16(wm0),
        "wv0": to_bf16(inputs["Wv0"]),
        "wms": to_bf16(wms),
        "wvs": to_bf16(inputs["Wvs"]),
        "wf": to_bf16(wf),
        "nws": to_bf16(-wf.sum(axis=1) / DM),
        "g1": np.ascontiguousarray(inputs["g1"], np.float32),
        "b1": np.ascontiguousarray(inputs["b1"], np.float32),
        "g2": np.ascontiguousarray(inputs["g2"], np.float32),
        "b2": np.ascontiguousarray(inputs["b2"], np.float32),
        "bf": np.ascontiguousarray(inputs["bf"], np.float32),
    }
    in_maps = []
    for i in range(NCORES):
        xt = to_bf16(
            np.concatenate([x[NB * i + b].T for b in range(NB)], axis=1))
        m = dict(shared)
        m["xt"] = xt
        in_maps.append(m)

    affine_identity = bool(
        np.all(shared["g1"] == 1) and np.all(shared["b1"] == 0)
        and np.all(shared["g2"] == 1) and np.all(shared["b2"] == 0))
    bf_zero = bool(np.all(shared["bf"] == 0))

    nc = bacc.Bacc()
    build_graph(nc, affine_identity=affine_identity, bf_zero=bf_zero)
    res = run_bass_kernel_spmd(nc, in_maps, list(range(NCORES)), trace=TRACE)
    if TRACE:
        print("exec_time_ns:", res.exec_time_ns, "mean:", res.mean_exec_time_ns)
        kernel.last_result = res

    y = np.empty((NCORES * NB, NT, DM), np.float32)
    for i in range(NCORES):
        o = res.results[i]["out"]
        for b in range(NB):
            y[NB * i + b] = o[b * NT:(b + 1) * NT]
    return y
